# revision 1
# baseline (speedup 1.0000x reference)
"""MoNet (GMM graph conv) 3-layer kernel for one TRN2 chip (8 NeuronCores).

Strategy (graph/data parallel, dst-sharded):
  - Nodes are split into 8 contiguous shards of 2500; core c owns all edges
    whose dst lands in its shard (host-side index prep only).
  - Per layer, each core:
      * computes Gaussian mixture weights w[e,k] on device (DVE+ACT),
      * dma_gather's h[src[e]] rows (bf16, 256B rows) from a replicated
        full-h DRAM table,
      * aggregates g_k[n] = sum_e 1[dst=n] * w[e,k] * h[src[e]] with a
        one-hot "mask matmul" on the tensor engine (PSUM accumulation over
        128-edge tiles, node-tile = 128 dst nodes),
      * applies the dense transform agg = sum_k g_k @ W_k + bias
        (PE transposes + 4 accumulated matmuls),
      * AllGather's the new h shard to every core (bf16).
  - Compute dtype bf16 (fp32 PSUM accumulation); w computed in fp32.
"""

import sys

sys.path.insert(0, "/opt/trn_rl_repo")

import numpy as np
import ml_dtypes

from concourse import bacc, mybir
from concourse import tile
from concourse.bass_utils import run_bass_kernel_spmd
from concourse.library_config import mlp

import os
N_LAYERS = int(os.environ.get("KERN_LAYERS", "3"))
USE_CC = os.environ.get("KERN_CC", "1") == "1"
N_BINS = int(os.environ.get("KERN_BINS", "99"))
SKIP_W = os.environ.get("KERN_SKIPW", "0") == "1"
GSPLIT = os.environ.get("KERN_GSPLIT", "0") == "1"

N_NODES = 20000
N_EDGES = 320000
IN_FEATS = 64
D = 128            # padded feature width, = hidden width for all layers
K = 4
N_CORES = 8
SHARD = N_NODES // N_CORES          # 2500
NT = (SHARD + 127) // 128           # 20 node tiles per core (last has 68 rows)
BF = mybir.dt.bfloat16
F32 = mybir.dt.float32
I16 = mybir.dt.int16
bf16 = ml_dtypes.bfloat16


def _plan_edges(src, dst):
    """Partition + sort + pad edges. Returns per-core index arrays and the
    shared per-node-tile tile counts T_bins (identical across cores so the
    single SPMD program fits every core)."""
    core_of = dst // SHARD
    plans = []
    counts = np.zeros((N_CORES, NT), dtype=np.int64)
    per_core = []
    for c in range(N_CORES):
        sel = np.nonzero(core_of == c)[0]
        dl = dst[sel] - c * SHARD
        nt = dl // 128
        order = np.argsort(nt, kind="stable")
        sel, dl, nt = sel[order], dl[order], nt[order]
        per_core.append((sel, dl, nt))
        counts[c] = np.bincount(nt, minlength=NT)
    T_bins = np.maximum(1, (counts.max(axis=0) + 127) // 128).astype(np.int64)
    T_tot = int(T_bins.sum())
    for c in range(N_CORES):
        sel, dl, nt = per_core[c]
        srcP = np.zeros(T_tot * 128, dtype=np.int64)
        dstlocP = np.full(T_tot * 128, -1.0, dtype=np.float32)
        origP = np.full(T_tot * 128, -1, dtype=np.int64)
        tbase = 0
        pos = 0
        for b in range(NT):
            n = int(counts[c, b])
            lo = tbase * 128
            srcP[lo : lo + n] = src[sel[pos : pos + n]]
            dstlocP[lo : lo + n] = (dl[pos : pos + n] - b * 128).astype(np.float32)
            origP[lo : lo + n] = sel[pos : pos + n]
            pos += n
            tbase += int(T_bins[b])
        plans.append((srcP, dstlocP, origP))
    return T_bins, T_tot, plans


def _wrap_idx(idx_flat):
    """[n] int -> [128, n//16] int16 gather-index layout (16-partition wrap,
    replicated across the 8 Q7 cores)."""
    n = idx_flat.shape[0]
    w = idx_flat.reshape(n // 16, 16).T.astype(np.int16)
    return np.tile(w, (8, 1)).copy()


def _rep(v, cols=None):
    """Replicate a scalar/vector across 128 partitions as float32."""
    v = np.asarray(v, dtype=np.float32).reshape(-1)
    return np.tile(v, (128, 1)).copy()


def build_program(T_bins, T_tot):
    nc = bacc.Bacc("TRN2", target_bir_lowering=False, debug=False,
                   num_devices=N_CORES)

    featP_d = nc.dram_tensor("featP", [128, T_tot, D], BF, kind="ExternalInput")
    idx_d = nc.dram_tensor("idx", [128, T_tot * 8], I16, kind="ExternalInput")
    dstloc_d = nc.dram_tensor("dstloc", [128, T_tot], BF, kind="ExternalInput")
    pseudo_d = nc.dram_tensor("pseudo", [128, T_tot, 2], F32, kind="ExternalInput")
    iota_d = nc.dram_tensor("iota", [128, 128], BF, kind="ExternalInput")
    ident_d = nc.dram_tensor("ident", [128, 128], BF, kind="ExternalInput")
    fcw_d, pw_d, pb_d, mu_d, isg_d, bias_d = [], [], [], [], [], []
    for l in range(3):
        fcw_d.append(nc.dram_tensor(f"fcw{l}", [128, K, D], BF, kind="ExternalInput"))
        pw_d.append(nc.dram_tensor(f"pw{l}", [128, 4], F32, kind="ExternalInput"))
        pb_d.append(nc.dram_tensor(f"pb{l}", [128, 2], F32, kind="ExternalInput"))
        mu_d.append(nc.dram_tensor(f"mu{l}", [128, 2 * K], F32, kind="ExternalInput"))
        isg_d.append(nc.dram_tensor(f"isg{l}", [128, 2 * K], F32, kind="ExternalInput"))
        bias_d.append(nc.dram_tensor(f"bias{l}", [128, D], F32, kind="ExternalInput"))
    out_d = nc.dram_tensor("out", [SHARD, D], F32, kind="ExternalOutput")

    AF = mybir.ActivationFunctionType
    OP = mybir.AluOpType

    with tile.TileContext(nc) as tc:
        with (
            tc.tile_pool(name="const", bufs=1) as cpool,
            tc.tile_pool(name="wrk", bufs=2) as wpool,
            tc.tile_pool(name="hbin", bufs=3) as hpool,
            tc.tile_pool(name="maskp", bufs=3) as mpool,
            tc.tile_pool(name="scp", bufs=4) as spool,
            tc.tile_pool(name="outp", bufs=3) as opool,
            tc.tile_pool(name="gps", bufs=2, space="PSUM") as gpsum,
            tc.tile_pool(name="tps", bufs=2, space="PSUM") as tpsum,
            tc.tile_pool(name="aps", bufs=2, space="PSUM") as apsum,
            tc.tile_pool(name="dram", bufs=1, space="DRAM") as dram,
        ):
            nc.gpsimd.load_library(mlp)

            idx_sb = cpool.tile([128, T_tot * 8], I16)
            dstloc = cpool.tile([128, T_tot], BF)
            pseudo = cpool.tile([128, T_tot, 2], F32)
            iota = cpool.tile([128, 128], BF)
            ident = cpool.tile([128, 128], BF)
            nc.sync.dma_start(idx_sb[:], idx_d[:])
            nc.sync.dma_start(dstloc[:], dstloc_d[:])
            nc.sync.dma_start(pseudo[:], pseudo_d[:])
            nc.sync.dma_start(iota[:], iota_d[:])
            nc.sync.dma_start(ident[:], ident_d[:])
            fcw, pwt, pbt, mut, isgt, biast = [], [], [], [], [], []
            for l in range(3):
                fcw.append(cpool.tile([128, K, D], BF, tag=f"fcw{l}", name=f"fcw{l}"))
                pwt.append(cpool.tile([128, 4], F32, tag=f"pw{l}", name=f"pwt{l}"))
                pbt.append(cpool.tile([128, 2], F32, tag=f"pb{l}", name=f"pbt{l}"))
                mut.append(cpool.tile([128, 2 * K], F32, tag=f"mu{l}", name=f"mut{l}"))
                isgt.append(cpool.tile([128, 2 * K], F32, tag=f"isg{l}", name=f"isgt{l}"))
                biast.append(cpool.tile([128, D], F32, tag=f"bias{l}", name=f"biast{l}"))
                nc.sync.dma_start(fcw[l][:], fcw_d[l][:])
                nc.sync.dma_start(pwt[l][:], pw_d[l][:])
                nc.sync.dma_start(pbt[l][:], pb_d[l][:])
                nc.sync.dma_start(mut[l][:], mu_d[l][:])
                nc.sync.dma_start(isgt[l][:], isg_d[l][:])
                nc.sync.dma_start(biast[l][:], bias_d[l][:])

            # DRAM bounce buffers for the inter-layer AllGather
            shard_t = [dram.tile([SHARD, D], BF, tag=f"shard{l}", name=f"shard{l}") for l in range(2)]
            hag_t = [dram.tile([N_NODES, D], BF, tag=f"hag{l}", name=f"hag{l}") for l in range(2)]

            for l in range(N_LAYERS):
                hsrc = None if (l == 0 or not USE_CC) else hag_t[l - 1][:]

                # ---- Phase W: mixture weights w[e,k] for every edge slot ----
                w_all = wpool.tile([128, K, T_tot], F32, tag="w_all")
                u = wpool.tile([128, 2, T_tot], F32, tag="u")
                tmp0 = wpool.tile([128, T_tot], F32, tag="tmp0")
                tmp1 = wpool.tile([128, T_tot], F32, tag="tmp1")
                if SKIP_W:
                    nc.vector.memset(w_all[:], 0.5)
                for d in range(2 if not SKIP_W else 0):
                    # u_d = tanh(p0*pw[0,d] + p1*pw[1,d] + pb[d])
                    nc.vector.tensor_scalar(tmp0[:], pseudo[:, :, 0],
                                            pwt[l][:, d : d + 1], None, OP.mult)
                    nc.vector.tensor_scalar(tmp1[:], pseudo[:, :, 1],
                                            pwt[l][:, 2 + d : 3 + d], None, OP.mult)
                    nc.vector.tensor_tensor(tmp0[:], tmp0[:], tmp1[:], OP.add)
                    nc.scalar.activation(u[:, d, :], tmp0[:], AF.Tanh,
                                         bias=pbt[l][:, d : d + 1])
                for k in range(K if not SKIP_W else 0):
                    nc.vector.tensor_scalar(tmp0[:], u[:, 0, :],
                                            mut[l][:, 2 * k : 2 * k + 1],
                                            isgt[l][:, 2 * k : 2 * k + 1],
                                            OP.subtract, OP.mult)
                    nc.vector.tensor_scalar(tmp1[:], u[:, 1, :],
                                            mut[l][:, 2 * k + 1 : 2 * k + 2],
                                            isgt[l][:, 2 * k + 1 : 2 * k + 2],
                                            OP.subtract, OP.mult)
                    nc.vector.tensor_tensor(tmp0[:], tmp0[:], tmp0[:], OP.mult)
                    nc.vector.tensor_tensor(tmp1[:], tmp1[:], tmp1[:], OP.mult)
                    nc.vector.tensor_tensor(tmp0[:], tmp0[:], tmp1[:], OP.add)
                    nc.scalar.activation(w_all[:, k, :], tmp0[:], AF.Exp, scale=-0.5)

                # ---- Phase E: per node-tile gather + mask-matmul + transform ----
                # gather chunks of CH tiles (dma_gather caps at 1024 idxs);
                # layer 0 reads host-pre-gathered rows contiguously instead.
                CH = 8
                chunks = {}

                def get_chunk(t):
                    c = t // CH
                    if c not in chunks:
                        n = min(CH, T_tot - c * CH)
                        Hc = hpool.tile([128, CH, D], BF, tag="hbin",
                                        name=f"hb_{l}_{c}")
                        if hsrc is None:
                            nc.sync.dma_start(Hc[:, :n, :],
                                              featP_d[:, c * CH : c * CH + n, :])
                        else:
                            nc.gpsimd.dma_gather(
                                Hc[:, :n, :], hsrc,
                                idx_sb[:, c * CH * 8 : (c * CH + n) * 8],
                                num_idxs=n * 128, num_idxs_reg=n * 128, elem_size=D,
                            )
                        # scale all 4 k-blocks for the whole chunk (batched,
                        # 1-port DVE TTs for k0/k1, ACT copy-scale for k2/k3)
                        sC = spool.tile([128, K, CH, D], BF, tag="sc",
                                        name=f"sc_{l}_{c}")
                        for k in range(2):
                            nc.vector.tensor_tensor(
                                sC[:, k, :n, :], Hc[:, :n, :],
                                w_all[:, k, c * CH : c * CH + n].unsqueeze(2)
                                    .broadcast_to([128, n, D]),
                                OP.mult)
                        chunks[c] = (Hc, sC)
                    return chunks[c]

                tbase = 0
                for b in range(min(NT, N_BINS)):
                    Tn = int(T_bins[b])
                    mB = mpool.tile([128, Tn, 128], BF, tag="mask")
                    nc.vector.tensor_tensor(
                        mB[:],
                        iota[:].unsqueeze(1).broadcast_to([128, Tn, 128]),
                        dstloc[:, tbase : tbase + Tn].unsqueeze(2)
                            .broadcast_to([128, Tn, 128]),
                        OP.is_equal,
                    )
                    gp = gpsum.tile([128, K * D], F32, tag="g")
                    for j in range(Tn):
                        t = tbase + j
                        Hc, sC = get_chunk(t)
                        for k in range(2, K):
                            nc.scalar.activation(sC[:, k, t % CH, :],
                                                 Hc[:, t % CH, :], AF.Copy,
                                                 scale=w_all[:, k, t : t + 1])
                        nc.tensor.matmul(gp[:], mB[:, j, :],
                                         sC[:, :, t % CH, :],
                                         start=(j == 0), stop=(j == Tn - 1))
                    # transform: agg = sum_k g_k @ W_k  (+ bias)
                    gsb = opool.tile([128, K, D], BF, tag="gsb")
                    nc.scalar.activation(gsb[:].rearrange("p k d -> p (k d)"),
                                         gp[:], AF.Copy)
                    aggp = apsum.tile([128, D], F32, tag="agg")
                    for k in range(K):
                        gt_ps = tpsum.tile([128, 128], BF, tag="gt")
                        nc.tensor.transpose(gt_ps[:], gsb[:, k, :], ident[:])
                        gt_sb = opool.tile([128, 128], BF, tag="gtsb")
                        nc.vector.tensor_copy(gt_sb[:], gt_ps[:])
                        nc.tensor.matmul(aggp[:], gt_sb[:], fcw[l][:, k, :],
                                         start=(k == 0), stop=(k == 3))
                    rows = min(128, SHARD - b * 128)
                    if l < N_LAYERS - 1:
                        ht = opool.tile([128, D], BF, tag="hout")
                        nc.vector.tensor_tensor(ht[:], aggp[:], biast[l][:], OP.add)
                        nc.sync.dma_start(
                            shard_t[l][b * 128 : b * 128 + rows, :], ht[:rows, :])
                    else:
                        hf = opool.tile([128, D], F32, tag="hfin")
                        nc.vector.tensor_tensor(hf[:], aggp[:], biast[l][:], OP.add)
                        nc.sync.dma_start(
                            out_d[b * 128 : b * 128 + rows, :], hf[:rows, :])
                    tbase += Tn

                if l < 2 and USE_CC:
                    nc.gpsimd.collective_compute(
                        "AllGather", OP.bypass,
                        replica_groups=[list(range(N_CORES))],
                        ins=[shard_t[l].opt()], outs=[hag_t[l].opt()],
                    )
    nc.compile()
    return nc


def _host_inputs(inputs, T_bins, T_tot, plans):
    """Build the 8 per-core input maps."""
    feats = np.zeros((N_NODES, D), dtype=np.float32)
    feats[:, :IN_FEATS] = inputs["features"]
    feat_bf = feats.astype(bf16)
    iota = np.tile(np.arange(128, dtype=np.float32), (128, 1)).astype(bf16)
    ident = np.eye(128, dtype=np.float32).astype(bf16)

    common = {"iota": iota, "ident": ident}
    for l in range(3):
        fc = np.asarray(inputs[f"fc_w{l}"], dtype=np.float32)   # [din, K*128]
        fcp = np.zeros((D, K * D), dtype=np.float32)
        fcp[: fc.shape[0], :] = fc
        fcw = fcp.reshape(D, K, D).astype(bf16)                  # [j, k, o]
        common[f"fcw{l}"] = fcw
        pw = np.asarray(inputs[f"pw{l}"], dtype=np.float32)      # [2,2]
        common[f"pw{l}"] = _rep([pw[0, 0], pw[0, 1], pw[1, 0], pw[1, 1]])
        common[f"pb{l}"] = _rep(inputs[f"pb{l}"])
        common[f"mu{l}"] = _rep(np.asarray(inputs[f"mu{l}"]).reshape(-1))
        common[f"isg{l}"] = _rep(np.asarray(inputs[f"inv_sigma{l}"]).reshape(-1))
        common[f"bias{l}"] = _rep(inputs[f"bias{l}"])

    pseudo = np.asarray(inputs["pseudo"], dtype=np.float32)
    in_maps = []
    for c in range(N_CORES):
        srcP, dstlocP, origP = plans[c]
        m = dict(common)
        m["idx"] = _wrap_idx(srcP)
        # layer-0 source rows pre-gathered into edge order (input sharding)
        m["featP"] = (feat_bf[srcP].reshape(T_tot, 128, D)
                      .transpose(1, 0, 2).copy())
        m["dstloc"] = dstlocP.astype(bf16).reshape(T_tot, 128).T.copy()
        ps = np.zeros((T_tot * 128, 2), dtype=np.float32)
        valid = origP >= 0
        ps[valid] = pseudo[origP[valid]]
        m["pseudo"] = ps.reshape(T_tot, 128, 2).transpose(1, 0, 2).copy()
        in_maps.append(m)
    return in_maps


_CACHE = {}


def _get_compiled(src, dst):
    key = (src.tobytes(), dst.tobytes())
    h = hash(key)
    if h not in _CACHE:
        T_bins, T_tot, plans = _plan_edges(np.asarray(src, dtype=np.int64),
                                           np.asarray(dst, dtype=np.int64))
        nc = build_program(T_bins, T_tot)
        _CACHE[h] = (nc, T_bins, T_tot, plans)
    return _CACHE[h]


def run(inputs, trace=False, **kwargs):
    nc, T_bins, T_tot, plans = _get_compiled(
        np.asarray(inputs["src"]), np.asarray(inputs["dst"]))
    in_maps = _host_inputs(inputs, T_bins, T_tot, plans)
    res = run_bass_kernel_spmd(nc, in_maps, core_ids=list(range(N_CORES)),
                               trace=trace, **kwargs)
    out = np.concatenate([res.results[c]["out"] for c in range(N_CORES)], axis=0)
    return out.astype(np.float32), res


def kernel(**inputs):
    out, _ = run(inputs)
    return out



# revision 5
# speedup vs baseline: 1.1891x; 1.1891x over previous
"""MoNet (GMM graph conv) 3-layer kernel for one TRN2 chip (8 NeuronCores).

Strategy (graph/data parallel, dst-sharded, weighted-mask aggregation):
  - Nodes are LPT-packed into 160 balanced bins of <=128 nodes (~2000
    edges each); core c owns bins [20c, 20c+20) -> uniform 16 edge-tiles
    per bin, zero ragged tails (host-side index prep only).
  - Per layer, each core:
      * computes Gaussian mixture weights w[e,k] on device (DVE+ACT),
      * builds per-bin WEIGHTED one-hot masks wm[e,(k,dst)] = w[e,k] *
        1[dstloc[e]==dst] with batched 2x-mode DVE ops (w and dstloc are
        pre-replicated 8x along the free dim so no 0-stride operand),
      * aggregates gT[d,(k,dst)] = sum_e h[src[e],d] * wm[e,(k,dst)] on
        the tensor engine (h-tile stationary, wm moving, PSUM accum) --
        the transposed output feeds the dense transform directly:
        agg[dst,o] = sum_k gT_k^T @ W_k with no PE transposes,
      * gathers h[src[e]] rows (bf16, 256B) into a whole-layer SBUF
        landing zone via gpsimd dma_gather chunks,
      * AllGather's the new h shard to every core (bf16).
  - Compute dtype bf16 (fp32 PSUM accumulation); w computed in fp32.
"""

import os
import sys

sys.path.insert(0, "/opt/trn_rl_repo")

import numpy as np
import ml_dtypes

from concourse import bacc, mybir
from concourse import tile
from concourse.bass_utils import run_bass_kernel_spmd
from concourse.library_config import mlp

N_LAYERS = int(os.environ.get("KERN_LAYERS", "3"))
USE_CC = os.environ.get("KERN_CC", "1") == "1"
CH = int(os.environ.get("KERN_CH", "8"))          # gather chunk (tiles)
SHARED_AG = os.environ.get("KERN_SHARED", "0") == "1"

N_NODES = 20000
N_EDGES = 320000
IN_FEATS = 64
D = 128
K = 4
N_CORES = 8
BPC = 20                      # bins per core
GB = N_CORES * BPC            # 160 global bins
SHARD = BPC * 128             # 2560 rows per core
NTOT = GB * 128               # 20480 rows total
BF = mybir.dt.bfloat16
F32 = mybir.dt.float32
I16 = mybir.dt.int16
bf16 = ml_dtypes.bfloat16


def _plan(src, dst):
    """Balanced-bin edge partition. Returns (T_BIN, plans, node_gslot)."""
    deg = np.bincount(dst, minlength=N_NODES).astype(np.int64)
    order = np.argsort(-deg, kind="stable")
    import heapq
    heap = [(0, b) for b in range(GB)]
    heapq.heapify(heap)
    counts = np.zeros(GB, dtype=np.int64)
    loads = np.zeros(GB, dtype=np.int64)
    node_bin = np.empty(N_NODES, dtype=np.int64)
    node_slot = np.empty(N_NODES, dtype=np.int64)
    for n in order:
        while True:
            load, b = heapq.heappop(heap)
            if counts[b] < 128:
                break
        node_bin[n] = b
        node_slot[n] = counts[b]
        counts[b] += 1
        loads[b] = load + deg[n]
        if counts[b] < 128:
            heapq.heappush(heap, (loads[b], b))
    node_gslot = node_bin * 128 + node_slot          # row in the hag table

    T_BIN = int(np.max((loads + 127) // 128))
    T_tot = BPC * T_BIN

    ebin = node_bin[dst]                              # global bin per edge
    eslot = node_slot[dst]
    esrc_g = node_gslot[src]                          # hag row of source
    plans = []
    for c in range(N_CORES):
        srcP = np.zeros(T_tot * 128, dtype=np.int64)      # hag rows
        srcO = np.zeros(T_tot * 128, dtype=np.int64)      # original node ids
        dstslotP = np.full(T_tot * 128, -1.0, dtype=np.float32)
        origP = np.full(T_tot * 128, -1, dtype=np.int64)
        for bl in range(BPC):
            g = c * BPC + bl
            sel = np.nonzero(ebin == g)[0]
            so = np.argsort(esrc_g[sel], kind="stable")   # src-sorted
            sel = sel[so]
            n = sel.shape[0]
            lo = bl * T_BIN * 128
            srcP[lo : lo + n] = esrc_g[sel]
            srcO[lo : lo + n] = src[sel]
            dstslotP[lo : lo + n] = eslot[sel].astype(np.float32)
            origP[lo : lo + n] = sel
        plans.append((srcP, srcO, dstslotP, origP))
    return T_BIN, plans, node_gslot


def _wrap_idx(idx_flat):
    """[n] int -> [128, n//16] int16 (16-partition wrap, replicated x8 Q7)."""
    n = idx_flat.shape[0]
    w = idx_flat.reshape(n // 16, 16).T.astype(np.int16)
    return np.tile(w, (8, 1)).copy()


def _rep(v):
    v = np.asarray(v, dtype=np.float32).reshape(-1)
    return np.tile(v, (128, 1)).copy()


def build_program(T_BIN):
    T_tot = BPC * T_BIN
    nc = bacc.Bacc("TRN2", target_bir_lowering=False, debug=False,
                   num_devices=N_CORES)

    featP_d = nc.dram_tensor("featP", [128, T_tot, IN_FEATS], BF, kind="ExternalInput")
    idx_d = nc.dram_tensor("idx", [128, T_tot * 8], I16, kind="ExternalInput")
    dstloc8_d = nc.dram_tensor("dstloc8", [128, T_tot, 8], BF, kind="ExternalInput")
    pseudo_d = nc.dram_tensor("pseudo", [128, T_tot, 2], F32, kind="ExternalInput")
    iota_d = nc.dram_tensor("iota", [128, 128], BF, kind="ExternalInput")
    fcw_d, pw_d, pb_d, mu_d, isg_d, bias_d = [], [], [], [], [], []
    for l in range(3):
        fcw_d.append(nc.dram_tensor(f"fcw{l}", [128, K, D], BF, kind="ExternalInput"))
        pw_d.append(nc.dram_tensor(f"pw{l}", [128, 4], F32, kind="ExternalInput"))
        pb_d.append(nc.dram_tensor(f"pb{l}", [128, 2], F32, kind="ExternalInput"))
        mu_d.append(nc.dram_tensor(f"mu{l}", [128, 2 * K], F32, kind="ExternalInput"))
        isg_d.append(nc.dram_tensor(f"isg{l}", [128, 2 * K], F32, kind="ExternalInput"))
        bias_d.append(nc.dram_tensor(f"bias{l}", [128, D], F32, kind="ExternalInput"))
    out_d = nc.dram_tensor("out", [SHARD, D], F32, kind="ExternalOutput")

    AF = mybir.ActivationFunctionType
    OP = mybir.AluOpType

    with tile.TileContext(nc) as tc:
        with (
            tc.tile_pool(name="const", bufs=1) as cpool,
            tc.tile_pool(name="wrk", bufs=2) as wpool,
            tc.tile_pool(name="fp", bufs=3) as fpool,
            tc.tile_pool(name="w8p", bufs=2) as w8pool,
            tc.tile_pool(name="mkp", bufs=2) as mkpool,
            tc.tile_pool(name="wmp", bufs=2) as wmpool,
            tc.tile_pool(name="outp", bufs=3) as opool,
            tc.tile_pool(name="gps", bufs=2, space="PSUM") as gpsum,
            tc.tile_pool(name="aps", bufs=2, space="PSUM") as apsum,
            tc.tile_pool(name="dram", bufs=1, space="DRAM") as dram,
        ):
            nc.gpsimd.load_library(mlp)

            idx_sb = cpool.tile([128, T_tot * 8], I16)
            dstloc8 = cpool.tile([128, T_tot, 8], BF)
            pseudo = cpool.tile([128, T_tot, 2], F32)
            iota = cpool.tile([128, 128], BF)
            land = cpool.tile([128, T_tot, D], BF)    # gathered h rows
            nc.sync.dma_start(idx_sb[:], idx_d[:])
            nc.sync.dma_start(dstloc8[:], dstloc8_d[:])
            nc.sync.dma_start(pseudo[:], pseudo_d[:])
            nc.sync.dma_start(iota[:], iota_d[:])
            fcw, pwt, pbt, mut, isgt, biast = [], [], [], [], [], []
            for l in range(3):
                fcw.append(cpool.tile([128, K, D], BF, tag=f"fcw{l}", name=f"fcw{l}"))
                pwt.append(cpool.tile([128, 4], F32, tag=f"pw{l}", name=f"pwt{l}"))
                pbt.append(cpool.tile([128, 2], F32, tag=f"pb{l}", name=f"pbt{l}"))
                mut.append(cpool.tile([128, 2 * K], F32, tag=f"mu{l}", name=f"mut{l}"))
                isgt.append(cpool.tile([128, 2 * K], F32, tag=f"isg{l}", name=f"isgt{l}"))
                biast.append(cpool.tile([128, D], F32, tag=f"bias{l}", name=f"biast{l}"))
                nc.sync.dma_start(fcw[l][:], fcw_d[l][:])
                nc.sync.dma_start(pwt[l][:], pw_d[l][:])
                nc.sync.dma_start(pbt[l][:], pb_d[l][:])
                nc.sync.dma_start(mut[l][:], mu_d[l][:])
                nc.sync.dma_start(isgt[l][:], isg_d[l][:])
                nc.sync.dma_start(biast[l][:], bias_d[l][:])

            shard_t = [dram.tile([SHARD, D], BF, tag=f"shard{l}", name=f"shard{l}")
                       for l in range(2)]
            ag_space = "Shared" if SHARED_AG else "Local"
            hag_t = [dram.tile([NTOT, D], BF, tag=f"hag{l}", name=f"hag{l}",
                               addr_space=ag_space) for l in range(2)]

            for l in range(N_LAYERS):
                # ---- Phase W: mixture weights w[e,k] for every edge slot ----
                w_all = wpool.tile([128, K, T_tot], F32, tag="w_all")
                u = wpool.tile([128, 2, T_tot], F32, tag="u")
                tmp0 = wpool.tile([128, T_tot], F32, tag="tmp0")
                tmp1 = wpool.tile([128, T_tot], F32, tag="tmp1")
                for dd in range(2):
                    nc.vector.tensor_scalar(tmp0[:], pseudo[:, :, 0],
                                            pwt[l][:, dd : dd + 1], None, OP.mult)
                    nc.vector.tensor_scalar(tmp1[:], pseudo[:, :, 1],
                                            pwt[l][:, 2 + dd : 3 + dd], None, OP.mult)
                    nc.vector.tensor_tensor(tmp0[:], tmp0[:], tmp1[:], OP.add)
                    nc.scalar.activation(u[:, dd, :], tmp0[:], AF.Tanh,
                                         bias=pbt[l][:, dd : dd + 1])
                for k in range(K):
                    nc.vector.tensor_scalar(tmp0[:], u[:, 0, :],
                                            mut[l][:, 2 * k : 2 * k + 1],
                                            isgt[l][:, 2 * k : 2 * k + 1],
                                            OP.subtract, OP.mult)
                    nc.vector.tensor_scalar(tmp1[:], u[:, 1, :],
                                            mut[l][:, 2 * k + 1 : 2 * k + 2],
                                            isgt[l][:, 2 * k + 1 : 2 * k + 2],
                                            OP.subtract, OP.mult)
                    nc.vector.tensor_tensor(tmp0[:], tmp0[:], tmp0[:], OP.mult)
                    nc.vector.tensor_tensor(tmp1[:], tmp1[:], tmp1[:], OP.mult)
                    nc.vector.tensor_tensor(tmp0[:], tmp0[:], tmp1[:], OP.add)
                    nc.scalar.activation(w_all[:, k, :], tmp0[:], AF.Exp, scale=-0.5)

                # ---- gathers (layers>=1) / featP streaming (layer 0) ----
                din = IN_FEATS if l == 0 else D
                chunks = {}

                def get_chunk(t, l=l, din=din):
                    c = t // CH
                    if c not in chunks:
                        n = min(CH, T_tot - c * CH)
                        if l == 0:
                            Hc = fpool.tile([128, CH, IN_FEATS], BF, tag="fchunk",
                                            name=f"fc_{l}_{c}")
                            nc.sync.dma_start(Hc[:, :n, :],
                                              featP_d[:, c * CH : c * CH + n, :])
                            chunks[c] = Hc
                        else:
                            nc.gpsimd.dma_gather(
                                land[:, c * CH : c * CH + n, :], hag_t[l - 1][:],
                                idx_sb[:, c * CH * 8 : (c * CH + n) * 8],
                                num_idxs=n * 128, num_idxs_reg=n * 128, elem_size=D,
                            )
                            chunks[c] = None
                    return chunks[c]

                # ---- Phase E: per-bin weighted masks + aggregation ----
                for b in range(BPC):
                    bs = b * T_BIN
                    # w8: [128, K, T_BIN, 8] bf16 (w replicated 8x)
                    w8 = w8pool.tile([128, K, T_BIN, 8], BF, tag="w8")
                    nc.vector.tensor_copy(
                        w8[:],
                        w_all[:, :, bs : bs + T_BIN].unsqueeze(3)
                             .broadcast_to([128, K, T_BIN, 8]))
                    # mask: [128, T_BIN, 16, 8] == (iota == dstloc)
                    mB = mkpool.tile([128, T_BIN, 128], BF, tag="mask")
                    nc.vector.tensor_tensor(
                        mB[:].rearrange("p t (r j) -> p t r j", r=16),
                        iota[:].rearrange("p (r j) -> p r j", r=16)
                               .unsqueeze(1).broadcast_to([128, T_BIN, 16, 8]),
                        dstloc8[:, bs : bs + T_BIN, :].unsqueeze(2)
                               .broadcast_to([128, T_BIN, 16, 8]),
                        OP.is_equal,
                    )
                    # wm[e, t, k, dst] = mask[e, t, dst] * w[e, k, t]
                    wm = wmpool.tile([128, T_BIN, K, 128], BF, tag="wm")
                    for k in range(K):
                        nc.vector.tensor_tensor(
                            wm[:, :, k, :].rearrange("p t (r j) -> p t r j", r=16),
                            mB[:].rearrange("p t (r j) -> p t r j", r=16),
                            w8[:, k, :, :].unsqueeze(2)
                                .broadcast_to([128, T_BIN, 16, 8]),
                            OP.mult,
                        )
                    # aggregation: gT[d, (k, dst)] += h_tile^T-free matmul
                    gp = gpsum.tile([128, K * D], F32, tag="g")
                    for t in range(T_BIN):
                        tt = bs + t
                        Hc = get_chunk(tt)
                        if l == 0:
                            stat = Hc[:, tt % CH, :]
                        else:
                            stat = land[:, tt, :]
                        nc.tensor.matmul(gp[:din, :], stat,
                                         wm[:, t, :, :].rearrange("p k d -> p (k d)"),
                                         start=(t == 0), stop=(t == T_BIN - 1))
                    # transform: agg[dst, o] = sum_k gT_k^T @ W_k
                    gsb = opool.tile([128, K, D], BF, tag="gsb")
                    nc.scalar.activation(
                        gsb[:din, :, :].rearrange("p k d -> p (k d)"),
                        gp[:din, :], AF.Copy)
                    aggp = apsum.tile([128, D], F32, tag="agg")
                    for k in range(K):
                        nc.tensor.matmul(aggp[:], gsb[:din, k, :],
                                         fcw[l][:din, k, :],
                                         start=(k == 0), stop=(k == 3))
                    if l < N_LAYERS - 1:
                        ht = opool.tile([128, D], BF, tag="hout")
                        nc.vector.tensor_tensor(ht[:], aggp[:], biast[l][:], OP.add)
                        nc.sync.dma_start(shard_t[l][b * 128 : (b + 1) * 128, :],
                                          ht[:])
                    else:
                        hf = opool.tile([128, D], F32, tag="hfin")
                        nc.vector.tensor_tensor(hf[:], aggp[:], biast[l][:], OP.add)
                        nc.sync.dma_start(out_d[b * 128 : (b + 1) * 128, :], hf[:])

                if l < 2 and USE_CC:
                    nc.gpsimd.collective_compute(
                        "AllGather", OP.bypass,
                        replica_groups=[list(range(N_CORES))],
                        ins=[shard_t[l].opt()], outs=[hag_t[l].opt()],
                    )
    nc.compile()
    return nc


def _host_inputs(inputs, T_BIN, plans):
    T_tot = BPC * T_BIN
    feats = np.asarray(inputs["features"], dtype=np.float32).astype(bf16)
    iota = np.tile(np.arange(128, dtype=np.float32), (128, 1)).astype(bf16)

    common = {"iota": iota}
    for l in range(3):
        fc = np.asarray(inputs[f"fc_w{l}"], dtype=np.float32)   # [din, K*128]
        fcp = np.zeros((D, K * D), dtype=np.float32)
        fcp[: fc.shape[0], :] = fc
        common[f"fcw{l}"] = fcp.reshape(D, K, D).astype(bf16)
        pw = np.asarray(inputs[f"pw{l}"], dtype=np.float32)
        common[f"pw{l}"] = _rep([pw[0, 0], pw[0, 1], pw[1, 0], pw[1, 1]])
        common[f"pb{l}"] = _rep(inputs[f"pb{l}"])
        common[f"mu{l}"] = _rep(np.asarray(inputs[f"mu{l}"]).reshape(-1))
        common[f"isg{l}"] = _rep(np.asarray(inputs[f"inv_sigma{l}"]).reshape(-1))
        common[f"bias{l}"] = _rep(inputs[f"bias{l}"])

    pseudo = np.asarray(inputs["pseudo"], dtype=np.float32)
    in_maps = []
    for c in range(N_CORES):
        srcP, srcO, dstslotP, origP = plans[c]
        m = dict(common)
        m["idx"] = _wrap_idx(srcP)
        m["featP"] = (feats[srcO].reshape(T_tot, 128, IN_FEATS)
                      .transpose(1, 0, 2).copy())
        d8 = dstslotP.reshape(T_tot, 128).T.astype(bf16)          # [128, T]
        m["dstloc8"] = np.repeat(d8[:, :, None], 8, axis=2).copy()
        ps = np.zeros((T_tot * 128, 2), dtype=np.float32)
        valid = origP >= 0
        ps[valid] = pseudo[origP[valid]]
        m["pseudo"] = ps.reshape(T_tot, 128, 2).transpose(1, 0, 2).copy()
        in_maps.append(m)
    return in_maps


_CACHE = {}


def _get_compiled(src, dst):
    h = hash((src.tobytes(), dst.tobytes()))
    if h not in _CACHE:
        T_BIN, plans, node_gslot = _plan(np.asarray(src, dtype=np.int64),
                                         np.asarray(dst, dtype=np.int64))
        nc = build_program(T_BIN)
        _CACHE[h] = (nc, T_BIN, plans, node_gslot)
    return _CACHE[h]


def run(inputs, trace=False, **kwargs):
    nc, T_BIN, plans, node_gslot = _get_compiled(
        np.asarray(inputs["src"]), np.asarray(inputs["dst"]))
    in_maps = _host_inputs(inputs, T_BIN, plans)
    res = run_bass_kernel_spmd(nc, in_maps, core_ids=list(range(N_CORES)),
                               trace=trace, **kwargs)
    cat = np.concatenate([res.results[c]["out"] for c in range(N_CORES)], axis=0)
    out = cat[node_gslot].astype(np.float32)
    return out, res


def kernel(**inputs):
    out, _ = run(inputs)
    return out


# revision 31
# speedup vs baseline: 1.1901x; 1.0009x over previous
"""MoNet (GMM graph conv) 3-layer kernel for one TRN2 chip (8 NeuronCores).

V1 as HW-measured (999231 ns): balanced bins, weighted-mask aggregation,
plain post-AllGather gathers. Kept as a known-good fallback.
"""

import os
import sys

sys.path.insert(0, "/opt/trn_rl_repo")

import numpy as np
import ml_dtypes

from concourse import bacc, mybir
from concourse import tile
from concourse.bass_utils import run_bass_kernel_spmd
from concourse.library_config import mlp

N_LAYERS = int(os.environ.get("KERN_LAYERS", "3"))
USE_CC = os.environ.get("KERN_CC", "1") == "1"
CH = int(os.environ.get("KERN_CH", "8"))

N_NODES = 20000
N_EDGES = 320000
IN_FEATS = 64
D = 128
K = 4
N_CORES = 8
BPC = 20
GB = N_CORES * BPC
SHARD = BPC * 128
NTOT = GB * 128
BF = mybir.dt.bfloat16
F32 = mybir.dt.float32
I16 = mybir.dt.int16
bf16 = ml_dtypes.bfloat16


def _plan(src, dst):
    deg = np.bincount(dst, minlength=N_NODES).astype(np.int64)
    order = np.argsort(-deg, kind="stable")
    import heapq
    heap = [(0, b) for b in range(GB)]
    heapq.heapify(heap)
    counts = np.zeros(GB, dtype=np.int64)
    loads = np.zeros(GB, dtype=np.int64)
    node_bin = np.empty(N_NODES, dtype=np.int64)
    node_slot = np.empty(N_NODES, dtype=np.int64)
    for n in order:
        while True:
            load, b = heapq.heappop(heap)
            if counts[b] < 128:
                break
        node_bin[n] = b
        node_slot[n] = counts[b]
        counts[b] += 1
        loads[b] = load + deg[n]
        if counts[b] < 128:
            heapq.heappush(heap, (loads[b], b))
    node_gslot = node_bin * 128 + node_slot

    T_BIN = int(np.max((loads + 127) // 128))
    T_tot = BPC * T_BIN

    ebin = node_bin[dst]
    eslot = node_slot[dst]
    esrc_g = node_gslot[src]
    plans = []
    for c in range(N_CORES):
        srcP = np.zeros(T_tot * 128, dtype=np.int64)
        srcO = np.zeros(T_tot * 128, dtype=np.int64)
        dstslotP = np.full(T_tot * 128, -1.0, dtype=np.float32)
        origP = np.full(T_tot * 128, -1, dtype=np.int64)
        for bl in range(BPC):
            g = c * BPC + bl
            sel = np.nonzero(ebin == g)[0]
            so = np.argsort(esrc_g[sel], kind="stable")
            sel = sel[so]
            n = sel.shape[0]
            lo = bl * T_BIN * 128
            srcP[lo : lo + n] = esrc_g[sel]
            srcO[lo : lo + n] = src[sel]
            dstslotP[lo : lo + n] = eslot[sel].astype(np.float32)
            origP[lo : lo + n] = sel
        plans.append((srcP, srcO, dstslotP, origP))
    return T_BIN, plans, node_gslot


def _wrap_idx(idx_flat):
    n = idx_flat.shape[0]
    w = idx_flat.reshape(n // 16, 16).T.astype(np.int16)
    return np.tile(w, (8, 1)).copy()


def _rep(v):
    v = np.asarray(v, dtype=np.float32).reshape(-1)
    return np.tile(v, (128, 1)).copy()


def build_program(T_BIN):
    T_tot = BPC * T_BIN
    nc = bacc.Bacc("TRN2", target_bir_lowering=False, debug=False,
                   num_devices=N_CORES)

    featP_d = nc.dram_tensor("featP", [128, T_tot, IN_FEATS], BF, kind="ExternalInput")
    idx_d = nc.dram_tensor("idx", [128, T_tot * 8], I16, kind="ExternalInput")
    dstloc8_d = nc.dram_tensor("dstloc8", [128, T_tot, 8], BF, kind="ExternalInput")
    pseudo_d = nc.dram_tensor("pseudo", [128, T_tot, 2], F32, kind="ExternalInput")
    iota_d = nc.dram_tensor("iota", [128, 128], BF, kind="ExternalInput")
    fcw_d, pw_d, pb_d, mu_d, isg_d, bias_d = [], [], [], [], [], []
    for l in range(3):
        fcw_d.append(nc.dram_tensor(f"fcw{l}", [128, K, D], BF, kind="ExternalInput"))
        pw_d.append(nc.dram_tensor(f"pw{l}", [128, 4], F32, kind="ExternalInput"))
        pb_d.append(nc.dram_tensor(f"pb{l}", [128, 2], F32, kind="ExternalInput"))
        mu_d.append(nc.dram_tensor(f"mu{l}", [128, 2 * K], F32, kind="ExternalInput"))
        isg_d.append(nc.dram_tensor(f"isg{l}", [128, 2 * K], F32, kind="ExternalInput"))
        bias_d.append(nc.dram_tensor(f"bias{l}", [128, D], F32, kind="ExternalInput"))
    out_d = nc.dram_tensor("out", [SHARD, D], F32, kind="ExternalOutput")

    AF = mybir.ActivationFunctionType
    OP = mybir.AluOpType

    with tile.TileContext(nc) as tc:
        with (
            tc.tile_pool(name="const", bufs=1) as cpool,
            tc.tile_pool(name="wrk", bufs=2) as wpool,
            tc.tile_pool(name="fp", bufs=3) as fpool,
            tc.tile_pool(name="w8p", bufs=2) as w8pool,
            tc.tile_pool(name="mkp", bufs=2) as mkpool,
            tc.tile_pool(name="wmp", bufs=2) as wmpool,
            tc.tile_pool(name="outp", bufs=3) as opool,
            tc.tile_pool(name="gps", bufs=2, space="PSUM") as gpsum,
            tc.tile_pool(name="aps", bufs=2, space="PSUM") as apsum,
            tc.tile_pool(name="dram", bufs=1, space="DRAM") as dram,
        ):
            nc.gpsimd.load_library(mlp)

            idx_sb = cpool.tile([128, T_tot * 8], I16)
            dstloc8 = cpool.tile([128, T_tot, 8], BF)
            pseudo = cpool.tile([128, T_tot, 2], F32)
            iota = cpool.tile([128, 128], BF)
            land = cpool.tile([128, T_tot, D], BF)
            nc.sync.dma_start(idx_sb[:], idx_d[:])
            nc.sync.dma_start(dstloc8[:], dstloc8_d[:])
            nc.sync.dma_start(pseudo[:], pseudo_d[:])
            nc.sync.dma_start(iota[:], iota_d[:])
            fcw, pwt, pbt, mut, isgt, biast = [], [], [], [], [], []
            for l in range(3):
                fcw.append(cpool.tile([128, K, D], BF, tag=f"fcw{l}", name=f"fcw{l}"))
                pwt.append(cpool.tile([128, 4], F32, tag=f"pw{l}", name=f"pwt{l}"))
                pbt.append(cpool.tile([128, 2], F32, tag=f"pb{l}", name=f"pbt{l}"))
                mut.append(cpool.tile([128, 2 * K], F32, tag=f"mu{l}", name=f"mut{l}"))
                isgt.append(cpool.tile([128, 2 * K], F32, tag=f"isg{l}", name=f"isgt{l}"))
                biast.append(cpool.tile([128, D], F32, tag=f"bias{l}", name=f"biast{l}"))
                nc.sync.dma_start(fcw[l][:], fcw_d[l][:])
                nc.sync.dma_start(pwt[l][:], pw_d[l][:])
                nc.sync.dma_start(pbt[l][:], pb_d[l][:])
                nc.sync.dma_start(mut[l][:], mu_d[l][:])
                nc.sync.dma_start(isgt[l][:], isg_d[l][:])
                nc.sync.dma_start(biast[l][:], bias_d[l][:])

            shard_t = [dram.tile([SHARD, D], BF, tag=f"shard{l}", name=f"shard{l}")
                       for l in range(2)]
            hag_t = [dram.tile([NTOT, D], BF, tag=f"hag{l}", name=f"hag{l}")
                     for l in range(2)]

            for l in range(N_LAYERS):
                w_all = wpool.tile([128, K, T_tot], F32, tag="w_all")
                u = wpool.tile([128, 2, T_tot], F32, tag="u")
                tmp0 = wpool.tile([128, T_tot], F32, tag="tmp0")
                tmp1 = wpool.tile([128, T_tot], F32, tag="tmp1")
                for dd in range(2):
                    nc.vector.tensor_scalar(tmp0[:], pseudo[:, :, 0],
                                            pwt[l][:, dd : dd + 1], None, OP.mult)
                    nc.vector.tensor_scalar(tmp1[:], pseudo[:, :, 1],
                                            pwt[l][:, 2 + dd : 3 + dd], None, OP.mult)
                    nc.vector.tensor_tensor(tmp0[:], tmp0[:], tmp1[:], OP.add)
                    nc.scalar.activation(u[:, dd, :], tmp0[:], AF.Tanh,
                                         bias=pbt[l][:, dd : dd + 1])
                for k in range(K):
                    nc.vector.tensor_scalar(tmp0[:], u[:, 0, :],
                                            mut[l][:, 2 * k : 2 * k + 1],
                                            isgt[l][:, 2 * k : 2 * k + 1],
                                            OP.subtract, OP.mult)
                    nc.vector.tensor_scalar(tmp1[:], u[:, 1, :],
                                            mut[l][:, 2 * k + 1 : 2 * k + 2],
                                            isgt[l][:, 2 * k + 1 : 2 * k + 2],
                                            OP.subtract, OP.mult)
                    nc.vector.tensor_tensor(tmp0[:], tmp0[:], tmp0[:], OP.mult)
                    nc.vector.tensor_tensor(tmp1[:], tmp1[:], tmp1[:], OP.mult)
                    nc.vector.tensor_tensor(tmp0[:], tmp0[:], tmp1[:], OP.add)
                    nc.scalar.activation(w_all[:, k, :], tmp0[:], AF.Exp, scale=-0.5)

                din = IN_FEATS if l == 0 else D
                chunks = {}

                def get_chunk(t, l=l, din=din):
                    c = t // CH
                    if c not in chunks:
                        n = min(CH, T_tot - c * CH)
                        if l == 0:
                            Hc = fpool.tile([128, CH, IN_FEATS], BF, tag="fchunk",
                                            name=f"fc_{l}_{c}")
                            nc.sync.dma_start(Hc[:, :n, :],
                                              featP_d[:, c * CH : c * CH + n, :])
                            chunks[c] = Hc
                        else:
                            nc.gpsimd.dma_gather(
                                land[:, c * CH : c * CH + n, :], hag_t[l - 1][:],
                                idx_sb[:, c * CH * 8 : (c * CH + n) * 8],
                                num_idxs=n * 128, num_idxs_reg=n * 128, elem_size=D,
                            )
                            chunks[c] = None
                    return chunks[c]

                for b in range(BPC):
                    bs = b * T_BIN
                    w8 = w8pool.tile([128, K, T_BIN, 8], BF, tag="w8")
                    nc.vector.tensor_copy(
                        w8[:],
                        w_all[:, :, bs : bs + T_BIN].unsqueeze(3)
                             .broadcast_to([128, K, T_BIN, 8]))
                    mB = mkpool.tile([128, T_BIN, 128], BF, tag="mask")
                    nc.vector.tensor_tensor(
                        mB[:].rearrange("p t (r j) -> p t r j", r=16),
                        iota[:].rearrange("p (r j) -> p r j", r=16)
                               .unsqueeze(1).broadcast_to([128, T_BIN, 16, 8]),
                        dstloc8[:, bs : bs + T_BIN, :].unsqueeze(2)
                               .broadcast_to([128, T_BIN, 16, 8]),
                        OP.is_equal,
                    )
                    wm = wmpool.tile([128, T_BIN, K, 128], BF, tag="wm")
                    for k in range(K):
                        nc.vector.tensor_tensor(
                            wm[:, :, k, :].rearrange("p t (r j) -> p t r j", r=16),
                            mB[:].rearrange("p t (r j) -> p t r j", r=16),
                            w8[:, k, :, :].unsqueeze(2)
                                .broadcast_to([128, T_BIN, 16, 8]),
                            OP.mult,
                        )
                    gp = gpsum.tile([128, K * D], F32, tag="g")
                    for t in range(T_BIN):
                        tt = bs + t
                        Hc = get_chunk(tt)
                        if l == 0:
                            stat = Hc[:, tt % CH, :]
                        else:
                            stat = land[:, tt, :]
                        nc.tensor.matmul(gp[:din, :], stat,
                                         wm[:, t, :, :].rearrange("p k d -> p (k d)"),
                                         start=(t == 0), stop=(t == T_BIN - 1))
                    gsb = opool.tile([128, K, D], BF, tag="gsb")
                    nc.scalar.activation(
                        gsb[:din, :, :].rearrange("p k d -> p (k d)"),
                        gp[:din, :], AF.Copy)
                    aggp = apsum.tile([128, D], F32, tag="agg")
                    for k in range(K):
                        nc.tensor.matmul(aggp[:], gsb[:din, k, :],
                                         fcw[l][:din, k, :],
                                         start=(k == 0), stop=(k == 3))
                    if l < N_LAYERS - 1:
                        ht = opool.tile([128, D], BF, tag="hout")
                        nc.vector.tensor_tensor(ht[:], aggp[:], biast[l][:], OP.add)
                        nc.sync.dma_start(shard_t[l][b * 128 : (b + 1) * 128, :],
                                          ht[:])
                    else:
                        hf = opool.tile([128, D], F32, tag="hfin")
                        nc.vector.tensor_tensor(hf[:], aggp[:], biast[l][:], OP.add)
                        nc.sync.dma_start(out_d[b * 128 : (b + 1) * 128, :], hf[:])

                if l < 2 and USE_CC:
                    nc.gpsimd.collective_compute(
                        "AllGather", OP.bypass,
                        replica_groups=[list(range(N_CORES))],
                        ins=[shard_t[l].opt()], outs=[hag_t[l].opt()],
                    )
    nc.compile()
    return nc


def _host_inputs(inputs, T_BIN, plans):
    T_tot = BPC * T_BIN
    feats = np.asarray(inputs["features"], dtype=np.float32).astype(bf16)
    iota = np.tile(np.arange(128, dtype=np.float32), (128, 1)).astype(bf16)

    common = {"iota": iota}
    for l in range(3):
        fc = np.asarray(inputs[f"fc_w{l}"], dtype=np.float32)
        fcp = np.zeros((D, K * D), dtype=np.float32)
        fcp[: fc.shape[0], :] = fc
        common[f"fcw{l}"] = fcp.reshape(D, K, D).astype(bf16)
        pw = np.asarray(inputs[f"pw{l}"], dtype=np.float32)
        common[f"pw{l}"] = _rep([pw[0, 0], pw[0, 1], pw[1, 0], pw[1, 1]])
        common[f"pb{l}"] = _rep(inputs[f"pb{l}"])
        common[f"mu{l}"] = _rep(np.asarray(inputs[f"mu{l}"]).reshape(-1))
        common[f"isg{l}"] = _rep(np.asarray(inputs[f"inv_sigma{l}"]).reshape(-1))
        common[f"bias{l}"] = _rep(inputs[f"bias{l}"])

    pseudo = np.asarray(inputs["pseudo"], dtype=np.float32)
    in_maps = []
    for c in range(N_CORES):
        srcP, srcO, dstslotP, origP = plans[c]
        m = dict(common)
        m["idx"] = _wrap_idx(srcP)
        m["featP"] = (feats[srcO].reshape(T_tot, 128, IN_FEATS)
                      .transpose(1, 0, 2).copy())
        d8 = dstslotP.reshape(T_tot, 128).T.astype(bf16)
        m["dstloc8"] = np.repeat(d8[:, :, None], 8, axis=2).copy()
        ps = np.zeros((T_tot * 128, 2), dtype=np.float32)
        valid = origP >= 0
        ps[valid] = pseudo[origP[valid]]
        m["pseudo"] = ps.reshape(T_tot, 128, 2).transpose(1, 0, 2).copy()
        in_maps.append(m)
    return in_maps


_CACHE = {}


def _get_compiled(src, dst):
    h = hash((src.tobytes(), dst.tobytes()))
    if h not in _CACHE:
        T_BIN, plans, node_gslot = _plan(np.asarray(src, dtype=np.int64),
                                         np.asarray(dst, dtype=np.int64))
        nc = build_program(T_BIN)
        _CACHE[h] = (nc, T_BIN, plans, node_gslot)
    return _CACHE[h]


def run(inputs, trace=False, **kwargs):
    nc, T_BIN, plans, node_gslot = _get_compiled(
        np.asarray(inputs["src"]), np.asarray(inputs["dst"]))
    in_maps = _host_inputs(inputs, T_BIN, plans)
    res = run_bass_kernel_spmd(nc, in_maps, core_ids=list(range(N_CORES)),
                               trace=trace, **kwargs)
    cat = np.concatenate([res.results[c]["out"] for c in range(N_CORES)], axis=0)
    out = cat[node_gslot].astype(np.float32)
    return out, res


def kernel(**inputs):
    out, _ = run(inputs)
    return out
